# revision 10
# baseline (speedup 1.0000x reference)
"""Trainium2 kernel for nn_CCQC_classifier.

The reference applies a fixed 10-qubit/depth-5 circuit U (built only from the
tiny weight tensors) to each normalized, zero-padded input row, then reads out
logits l_k = <x|U^H Z_k U|x> / |x|^2 for k in {0,1} and returns mean NLL.

Since log_softmax over 2 classes depends only on the logit difference,
    nll_b = softplus((2*y_b - 1) * delta_b),   delta_b = x_b^T M_d x_b / |x_b|^2
with M_d = Re(U^H diag(z0 - z1) U)[:784, :784] a single fixed 784x784 real
symmetric matrix. The host builds M_d from the weights (cheap, data
independent); the device computes, per 1024-row batch shard:
    Y = X @ M_d              (bf16 matmul, fp32 accum, 2 waves over PSUM)
    e = rowsum(Y * X)        (one fused DVE scalar_tensor_tensor + accum)
    n2 = rowsum(X * X)       (one fused ACT Square + accum)
    delta = e / n2
    nll - ln2 = sgn*delta/2 + q(delta^2)   (even-part polynomial of softplus,
                                            degree 4, fit err < 6e-6)
    out[p] = sum_i nll_i     (128 partial sums per core)
Data-parallel across 8 NeuronCores; host adds ln2 back and averages.
"""

import sys

import numpy as np

for _p in ("/opt/trn_rl_repo", "/root/.axon_site/_ro/trn_rl_repo"):
    if _p not in sys.path:
        sys.path.append(_p)

N_QUBITS = 10
DEPTH = 5
DIM = 2**N_QUBITS  # 1024
F = 784  # true feature dim (rest of the 1024 state is zero padded)
B = 8192
NCORES = 8
BC = B // NCORES  # 1024 rows per core
P = 128
KT = 7  # contraction tiles: 784 padded to 896 = 7*128
NB = BC // P  # 8 batch chunks per core
FPAD = KT * P  # 896
WAVE1 = 4  # chunks 0..3 run k-outer (start while DMA streams in)

# softplus(x) - x/2 - ln2 is even: fit q(u) = p1 u + p2 u^2 + p3 u^3 + p4 u^4,
# u = x^2 in [0, 4.3] (|delta| <= 2 guaranteed: it is a difference of two
# Pauli-Z expectations). Least-squares fit, max abs err 5.5e-6.
_POLY = None  # computed lazily (tiny lstsq) and cached


def _softplus_poly():
    global _POLY
    if _POLY is None:
        u = np.linspace(0, 4.3, 20001)
        h = np.log(2 * np.cosh(np.sqrt(u) / 2)) - np.log(2.0)
        A = np.stack([u**k for k in range(1, 5)], axis=1)
        coef, *_ = np.linalg.lstsq(A, h, rcond=None)
        _POLY = [float(c) for c in coef]  # p1..p4
    return _POLY


# ---------------------------------------------------------------- host math
def _apply_1q(state, U, w):
    bdim = state.shape[0]
    s = state.reshape(bdim, 2**w, 2, 2 ** (N_QUBITS - 1 - w))
    s0 = s[:, :, 0, :].copy()
    s1 = s[:, :, 1, :].copy()
    s[:, :, 0, :] = U[0, 0] * s0 + U[0, 1] * s1
    s[:, :, 1, :] = U[1, 0] * s0 + U[1, 1] * s1
    return state


def _apply_c1q(state, U, ctrl, tgt):
    idx = np.arange(DIM)
    cbit = (idx >> (N_QUBITS - 1 - ctrl)) & 1
    tbit = (idx >> (N_QUBITS - 1 - tgt)) & 1
    tstride = 1 << (N_QUBITS - 1 - tgt)
    i0 = idx[(cbit == 1) & (tbit == 0)]
    i1 = i0 + tstride
    s0 = state[:, i0].copy()
    s1 = state[:, i1]
    state[:, i0] = U[0, 0] * s0 + U[0, 1] * s1
    state[:, i1] = U[1, 0] * s0 + U[1, 1] * s1
    return state


def _rx(t):
    c, s = np.cos(t / 2), np.sin(t / 2)
    return np.array([[c, -1j * s], [-1j * s, c]])


def _rz(t):
    e = np.exp(-1j * t / 2)
    return np.array([[e, 0], [0, np.conj(e)]])


def _build_Md(weights, weights_1, weights_2):
    """M_d = Re(U^H diag(z0-z1) U)[:784,:784] for the CCQC circuit."""
    weights = np.asarray(weights, np.float64)
    weights_1 = np.asarray(weights_1, np.float64)
    weights_2 = np.asarray(weights_2, np.float64)
    # state[b, :] = U @ e_b, so state = U^T as a matrix
    state = np.eye(DIM, dtype=np.complex128)
    for d in range(DEPTH):
        for i in range(N_QUBITS):
            state = _apply_1q(state, _rx(weights[d, i, 0]), i)
            state = _apply_1q(state, _rz(weights[d, i, 1]), i)
            state = _apply_1q(state, _rx(weights[d, i, 2]), i)
        r = 1 if d % 2 == 0 else 3
        for i in range(N_QUBITS):
            c = (i + r) % N_QUBITS
            state = _apply_c1q(state, _rz(weights[d, i, 3]), c, i)
            state = _apply_c1q(state, _rx(weights[d, i, 4]), c, i)
        state = _apply_1q(state, _rx(weights_1[d]), 0)
        state = _apply_1q(state, _rz(weights_2[d]), 0)
    # U[j, b] = state[b, j]
    idx = np.arange(DIM)
    zd = (2 * ((idx >> 8) & 1) - 2 * ((idx >> 9) & 1)).astype(np.float64)
    mask = zd != 0
    zsel = zd[mask]
    Ur = np.ascontiguousarray(state.real[:F, mask])
    Ui = np.ascontiguousarray(state.imag[:F, mask])
    Md = Ur @ (zsel[:, None] * Ur.T) + Ui @ (zsel[:, None] * Ui.T)
    return Md  # (784, 784) float64 symmetric


# ---------------------------------------------------------------- device code
_CACHE = {}


def _build_bass():
    import concourse.bacc as bacc
    import concourse.tile as tile
    from concourse import mybir

    f32 = mybir.dt.float32
    bf16 = mybir.dt.bfloat16
    MULT = mybir.AluOpType.mult
    ADD = mybir.AluOpType.add
    p1, p2, p3, p4 = _softplus_poly()

    nc = bacc.Bacc()
    xt_d = nc.dram_tensor("xt", (P, KT, BC), bf16, kind="ExternalInput")
    mb_d = nc.dram_tensor("mb", (P, KT, F), bf16, kind="ExternalInput")
    xb_d = nc.dram_tensor("xb", (P, NB, F), bf16, kind="ExternalInput")
    sgn_d = nc.dram_tensor("sgn", (P, NB), f32, kind="ExternalInput")
    out_d = nc.dram_tensor("out", (P, 1), f32, kind="ExternalOutput")

    NSPLITS = ((0, 512), (512, F))

    with tile.TileContext(nc) as tc:
        with (
            tc.tile_pool(name="const", bufs=1) as cpool,
            tc.tile_pool(name="scratch", bufs=3) as spool,
            tc.tile_pool(name="psum", bufs=4, space="PSUM") as psum,
        ):
            sgn = cpool.tile([P, NB], f32)
            nc.sync.dma_start(out=sgn[:], in_=sgn_d[:])
            # per-k-tile loads so wave-1 matmuls start as tiles arrive
            xt_k = [cpool.tile([P, BC], bf16, tag=f"xt{k}", name=f"xt{k}") for k in range(KT)]
            mb_k = [cpool.tile([P, F], bf16, tag=f"mb{k}", name=f"mb{k}") for k in range(KT)]
            xb_h = [cpool.tile([P, NB // 2, F], bf16, tag=f"xb{h}", name=f"xbh{h}") for h in (0, 1)]
            for k in range(KT):
                nc.sync.dma_start(out=mb_k[k][:], in_=mb_d[:, k, :])
                nc.sync.dma_start(out=xt_k[k][:], in_=xt_d[:, k, :])
                # xb halves land mid-stream: needed only when reductions begin
                if k == 3:
                    nc.sync.dma_start(out=xb_h[0][:], in_=xb_d[:, : NB // 2, :])
                if k == 5:
                    nc.sync.dma_start(out=xb_h[1][:], in_=xb_d[:, NB // 2 :, :])

            e = cpool.tile([P, NB], f32)
            n2 = cpool.tile([P, NB], f32)
            y_tiles = {}

            def xb_i(i):
                return xb_h[i // (NB // 2)][:, i % (NB // 2), :]

            def matmuls_for_chunk(i, k):
                for n0, n1 in NSPLITS:
                    nc.tensor.matmul(
                        y_tiles[i][:, n0:n1],
                        lhsT=xt_k[k][:, i * P : (i + 1) * P],
                        rhs=mb_k[k][:, n0:n1],
                        start=(k == 0),
                        stop=(k == KT - 1),
                    )

            def reduce_chunk(i):
                # n2[:, i] = rowsum(x^2) on ScalarE (fused square+accum)
                scr_a = spool.tile([P, F], f32, tag="scr_a")
                nc.scalar.activation(
                    out=scr_a[:],
                    in_=xb_i(i),
                    func=mybir.ActivationFunctionType.Square,
                    accum_out=n2[:, i : i + 1],
                )
                # e[:, i] = rowsum(Y * x): one fused DVE op
                scr_v = spool.tile([P, F], f32, tag="scr_v")
                nc.vector.scalar_tensor_tensor(
                    out=scr_v[:],
                    in0=y_tiles[i][:],
                    scalar=1.0,
                    in1=xb_i(i),
                    op0=MULT,
                    op1=MULT,
                    accum_out=e[:, i : i + 1],
                )

            # wave 1: chunks 0..3 k-outer -> PE starts on first (mb,xt) tiles
            for i in range(WAVE1):
                y_tiles[i] = psum.tile([P, F], f32, name=f"y{i}", tag="y")
            for k in range(KT):
                for i in range(WAVE1):
                    matmuls_for_chunk(i, k)
            for i in range(WAVE1):
                reduce_chunk(i)
            # wave 2: chunk-at-a-time so only the last chunk's reduce tails
            for i in range(WAVE1, NB):
                y_tiles[i] = psum.tile([P, F], f32, name=f"y{i}", tag="y")
                for k in range(KT):
                    matmuls_for_chunk(i, k)
                reduce_chunk(i)

            # delta = e/n2 ; nll - ln2 = sgn*delta/2 + q(delta^2)  (all DVE)
            rn2 = cpool.tile([P, NB], f32)
            nc.vector.reciprocal(out=rn2[:], in_=n2[:])
            d = cpool.tile([P, NB], f32)
            nc.vector.tensor_mul(d[:], e[:], rn2[:])
            u = cpool.tile([P, NB], f32)
            nc.vector.tensor_mul(u[:], d[:], d[:])
            v = cpool.tile([P, NB], f32)
            # v = (d * 0.5) * sgn
            nc.vector.scalar_tensor_tensor(
                out=v[:], in0=d[:], scalar=0.5, in1=sgn[:], op0=MULT, op1=MULT
            )
            # Horner-ish: acc = u*p4; acc = (acc+p3)*u; (acc+p2)*u; (acc+p1)*u
            acc = cpool.tile([P, NB], f32)
            nc.vector.tensor_scalar_mul(acc[:], u[:], p4)
            for c in (p3, p2, p1):
                nc.vector.scalar_tensor_tensor(
                    out=acc[:], in0=acc[:], scalar=c, in1=u[:], op0=ADD, op1=MULT
                )
            # w = acc + v with fused row-sum -> per-partition partials
            w = cpool.tile([P, NB], f32)
            nllp = cpool.tile([P, 1], f32)
            nc.vector.scalar_tensor_tensor(
                out=w[:],
                in0=acc[:],
                scalar=0.0,
                in1=v[:],
                op0=ADD,
                op1=ADD,
                accum_out=nllp[:],
            )
            nc.sync.dma_start(out=out_d[:], in_=nllp[:])

    nc.finalize()
    return nc


def kernel(x, y, weights, weights_1, weights_2):
    import ml_dtypes

    from concourse.bass_utils import run_bass_kernel_spmd

    x = np.asarray(x, np.float32)
    y = np.asarray(y)

    Md = _build_Md(weights, weights_1, weights_2)

    if "nc" not in _CACHE:
        _CACHE["nc"] = _build_bass()
    nc = _CACHE["nc"]

    bf16 = ml_dtypes.bfloat16
    # M_d padded to (896, 784) -> (P, KT, F): mb[p, k, :] = Md[k*128+p, :]
    Mpad = np.zeros((FPAD, F), np.float32)
    Mpad[:F] = Md.astype(np.float32)
    mb_host = np.ascontiguousarray(
        Mpad.reshape(KT, P, F).transpose(1, 0, 2).astype(bf16)
    )

    sgn_full = (2.0 * np.asarray(y, np.float64) - 1.0).astype(np.float32)

    in_maps = []
    for c in range(NCORES):
        xs = x[c * BC : (c + 1) * BC]  # (1024, 784)
        xsb = xs.astype(bf16)
        # xt[p, k, b] = x[b, k*128+p] ; zero-pad features 784..895
        xtt = np.ascontiguousarray(xsb.T)  # (784, 1024)
        xt3 = np.zeros((KT * P, BC), bf16)
        xt3[:F] = xtt
        xt_host = np.ascontiguousarray(xt3.reshape(KT, P, BC).transpose(1, 0, 2))
        # xb[p, i, :] = x[i*128+p, :]
        xb_host = np.ascontiguousarray(xsb.reshape(NB, P, F).transpose(1, 0, 2))
        # sgn[p, i] = 2*y[i*128+p]-1
        sg = sgn_full[c * BC : (c + 1) * BC]
        sgn_host = np.ascontiguousarray(sg.reshape(NB, P).T)
        in_maps.append(
            {"xt": xt_host, "mb": mb_host, "xb": xb_host, "sgn": sgn_host}
        )

    res = run_bass_kernel_spmd(nc, in_maps, core_ids=list(range(NCORES)))
    _CACHE["last"] = res  # test harness reads exec_time_ns/profile from here
    total = sum(float(r["out"].sum()) for r in res.results)
    return np.array(total / B + np.log(2.0), dtype=np.float32)
